# revision 4
# baseline (speedup 1.0000x reference)
"""CopyAttention (copy-generator head) Trainium2 kernel — vocab-sharded.

Computation (see reference):
  p_copy  = sigmoid(q @ W_copy + b_copy)                       [B,T,1]
  prob    = softmax(set_pad(q @ W_gen + b_gen))                [B,T,V]
  ori     = prob * (1 - p_copy)
  attn    = softmax(mask(qW_in @ mem^T))                       [B,T,S]
  copy    = (attn * p_copy) @ src_map                          [B,T,E]
  out     = concat([ori, copy], -1)                            [B,T,V+E]

Sharding (8 cores):
  - generator: column-parallel over the vocab. Core c owns W_gen columns
    [c*4000, (c+1)*4000), kept RESIDENT in SBUF (fp16, 64 KB/partition),
    and computes logits for ALL 2048 token rows, one 128-token tile at a
    time. Each token tile ends with a tiny [128] AllReduce of the
    per-row exp-sums across cores; the normalized f32 output tile then
    streams out while later tiles compute. This removes both the
    all-cores-read-all-of-W_gen DMA (8x less W traffic) and the
    output-scale tail (per-tile sums arrive during the run).
  - attention/copy path: data-parallel over tokens. Core c owns tokens
    [c*256, (c+1)*256) (batch c//2, half c%2), full fp32 matmuls
    (scores ~ N(0, 1024) are precision-critical), masked max-subtracted
    softmax, PE transpose, fp16 indicator matmul vs src_map.

Precision notes: generator softmax needs no max subtraction (logits ~
N(0,1)); exp stored f16 (2^-11); gen p_copy recomputed from fp16 q
(error ~1e-4, within f16 budget); PAD column handled by zeroing
W_gen[:,0] host-side (exp(0) == exp(-1e-20) == 1.0f exactly).
"""
import sys

if "/opt/trn_rl_repo" not in sys.path:
    sys.path.insert(0, "/opt/trn_rl_repo")

from contextlib import ExitStack

import numpy as np

import concourse.bass as bass
import concourse.bacc as bacc
import concourse.tile as tile
from concourse import mybir
from concourse.bass_utils import run_bass_kernel_spmd
from concourse.masks import make_identity

AF = mybir.ActivationFunctionType
ALU = mybir.AluOpType
F32 = mybir.dt.float32
F16 = mybir.dt.float16

B, T, D, S, V, E = 4, 512, 1024, 512, 32000, 512
P = 128
KC = D // P              # 8 contraction chunks
NTOK = B * T             # 2048 total tokens
NTT = NTOK // P          # 16 token tiles (gen path, per core: all of them)
TPC = 256                # tokens per core (attention path)
NT = TPC // P            # 2 token tiles per core (attention path)
VS = V // 8              # 4000 vocab columns per core
HW = 500                 # h-chunk width; 8 chunks of 500 = 4000
NH = VS // HW
NEG_INF = -1e18


def build(has_bgen: bool):
    nc = bacc.Bacc("TRN2", target_bir_lowering=False, debug=False, num_devices=8)

    qT16a = nc.dram_tensor("qT16a", [D, NTOK], F16, kind="ExternalInput")
    qT32 = nc.dram_tensor("qT32", [D, TPC], F32, kind="ExternalInput")
    w16s = nc.dram_tensor("w16s", [D, VS], F16, kind="ExternalInput")
    wc16 = nc.dram_tensor("wc16", [D, 1], F16, kind="ExternalInput")
    w_copy = nc.dram_tensor("w_copy", [D, 1], F32, kind="ExternalInput")
    b_copy = nc.dram_tensor("b_copy", [1], F32, kind="ExternalInput")
    w_in = nc.dram_tensor("w_in", [D, D], F32, kind="ExternalInput")
    memT = nc.dram_tensor("memT", [D, S], F32, kind="ExternalInput")
    smap = nc.dram_tensor("smap", [S, E], F16, kind="ExternalInput")
    maskadd = nc.dram_tensor("maskadd", [S], F32, kind="ExternalInput")
    bgen = (nc.dram_tensor("bgen", [VS], F16, kind="ExternalInput")
            if has_bgen else None)

    gen_out = nc.dram_tensor("gen_out", [NTOK, VS], F32, kind="ExternalOutput")
    copy_out = nc.dram_tensor("copy_out", [TPC, E], F32, kind="ExternalOutput")

    # collective buffers: per-token-tile [128] f32 row-sum partials
    cc_in = nc.dram_tensor("cc_in", [NTT, P], F32)
    cc_out = nc.dram_tensor("cc_out", [NTT, P], F32, addr_space="Shared")

    qT16a_r = qT16a.ap().rearrange("(c p) t -> p c t", p=P)
    qT32_r = qT32.ap().rearrange("(c p) t -> p c t", p=P)
    w16s_r = w16s.ap().rearrange("(c p) v -> p c v", p=P)
    wc16_r = wc16.ap().rearrange("(c p) o -> p c o", p=P)
    wc32_r = w_copy.ap().rearrange("(c p) o -> p c o", p=P)
    w_in_r = w_in.ap().rearrange("(c p) d -> p c d", p=P)
    memT_r = memT.ap().rearrange("(c p) s -> p c s", p=P)
    smap_r = smap.ap().rearrange("(c p) e -> p c e", p=P)

    with tile.TileContext(nc) as tc, ExitStack() as ctx:
        persist = ctx.enter_context(tc.tile_pool(name="persist", bufs=1))
        small = ctx.enter_context(tc.tile_pool(name="small", bufs=1))

        # ---- resident tensors ----
        wsh = persist.tile([P, KC, VS], F16)        # 64 KB/p: W_gen shard
        qT16a_t = persist.tile([P, KC, NTOK], F16)  # 32 KB/p: all-token q^T fp16
        qT32_t = persist.tile([P, KC, TPC], F32)    # 8 KB/p: token-shard q^T f32
        nc.sync.dma_start(wsh[:], w16s_r)
        nc.sync.dma_start(qT16a_t[:], qT16a_r)
        nc.sync.dma_start(qT32_t[:], qT32_r)
        if has_bgen:
            bg_t = small.tile([P, KC, HW], F16)     # broadcast b_gen shard
            nc.sync.dma_start(
                bg_t[:], bass.AP(tensor=bgen, offset=0, ap=[[0, P], [HW, KC], [1, HW]]))

        wc16_t = small.tile([P, KC, 1], F16)
        wc32_t = small.tile([P, KC, 1], F32)
        bc_t = small.tile([P, 1], F32)
        mask_t = small.tile([P, S], F32)
        smap_t = small.tile([P, S // P, E], F16)
        ident = small.tile([P, P], F16)
        nc.sync.dma_start(wc16_t[:], wc16_r)
        nc.sync.dma_start(wc32_t[:], wc32_r)
        nc.sync.dma_start(bc_t[:], bass.AP(tensor=b_copy, offset=0, ap=[[0, P], [1, 1]]))
        nc.sync.dma_start(mask_t[:], bass.AP(tensor=maskadd, offset=0, ap=[[0, P], [1, S]]))
        nc.sync.dma_start(smap_t[:], smap_r)
        make_identity(nc, ident[:])

        pc16 = small.tile([P, NTT], F32)     # p_copy for all tokens (fp16 path)
        ompc = small.tile([P, NTT], F32)     # 1 - p_copy
        totals = small.tile([P, NTT], F32)   # allreduced exp row sums
        cgen = small.tile([P, NTT], F32)     # (1-p_copy)/total
        psums = small.tile([P, NTT, 2], F32)  # per-half partial sums
        pc2 = small.tile([P, NT], F32)       # attention-path p_copy (f32)
        c2 = small.tile([P, NT], F32)
        asum = small.tile([P, NT], F32)

        # ---------------- p_copy for ALL tokens (fp16) ----------------
        with tc.tile_pool(name="pcp", bufs=2, space="PSUM") as pcp:
            for tt in range(NTT):
                tok = slice(tt * P, (tt + 1) * P)
                ps = pcp.tile([P, 1], F32, tag="pc")
                for k in range(KC):
                    nc.tensor.matmul(ps[:], qT16a_t[:, k, tok], wc16_t[:, k, :],
                                     start=(k == 0), stop=(k == KC - 1))
                nc.scalar.activation(pc16[:, tt:tt + 1], ps[:], AF.Sigmoid,
                                     bias=bc_t[:])
            nc.vector.tensor_scalar(ompc[:], pc16[:], -1.0, 1.0, ALU.mult, ALU.add)

        # ---------------- generator: per-token-tile rounds ----------------
        with tc.tile_pool(name="gps", bufs=1, space="PSUM") as gpsum, \
             tc.tile_pool(name="expp", bufs=3) as expp, \
             tc.tile_pool(name="stp", bufs=3) as stp:
            for tt in range(NTT):
                tok = slice(tt * P, (tt + 1) * P)
                exp_t = expp.tile([P, NH, HW], F16, tag="exp")
                for half in range(2):
                    ps = gpsum.tile([P, NH // 2, 512], F32, tag=f"ps{half}")
                    for h in range(NH // 2):
                        hh = half * (NH // 2) + h
                        for k in range(KC):
                            nc.tensor.matmul(
                                ps[:, h, :HW], qT16a_t[:, k, tok],
                                wsh[:, k, hh * HW:(hh + 1) * HW],
                                start=(k == 0), stop=(k == KC - 1))
                    if has_bgen:
                        for h in range(NH // 2):
                            hh = half * (NH // 2) + h
                            nc.vector.tensor_add(ps[:, h, :HW], ps[:, h, :HW],
                                                 bg_t[:, hh, :])
                    nc.scalar.activation(
                        exp_t[:, half * (NH // 2):(half + 1) * (NH // 2), :],
                        ps[:, :, :HW], AF.Exp,
                        accum_out=psums[:, tt, half:half + 1])
                # combine halves, ship partial sums through AllReduce
                nc.vector.tensor_add(psums[:, tt, 0:1], psums[:, tt, 0:1],
                                     psums[:, tt, 1:2])
                nc.sync.dma_start(cc_in.ap()[tt, :].unsqueeze(0),
                                  psums[:, tt, 0:1])
                nc.gpsimd.collective_compute(
                    "AllReduce", ALU.add,
                    replica_groups=[list(range(8))],
                    ins=[cc_in.ap()[tt, :].unsqueeze(0)],
                    outs=[cc_out.ap()[tt, :].unsqueeze(0)],
                )
                nc.sync.dma_start(totals[:, tt:tt + 1],
                                  cc_out.ap()[tt, :].unsqueeze(0))
                nc.vector.reciprocal(cgen[:, tt:tt + 1], totals[:, tt:tt + 1])
                nc.vector.tensor_mul(cgen[:, tt:tt + 1], cgen[:, tt:tt + 1],
                                     ompc[:, tt:tt + 1])
                for half in range(2):
                    st = stp.tile([P, NH // 2, HW], F32, tag="st")
                    nc.vector.tensor_scalar_mul(
                        st[:], exp_t[:, half * (NH // 2):(half + 1) * (NH // 2), :],
                        cgen[:, tt:tt + 1])
                    nc.sync.dma_start(
                        gen_out.ap()[tok, half * (VS // 2):(half + 1) * (VS // 2)],
                        st[:])

        # ---------------- attention path (fp32, token shard) ----------------
        attn16 = small.tile([P, NT, S], F16)
        attnT = small.tile([P, S // P, TPC], F16)
        qint = persist.tile([P, KC, TPC], F32)

        with tc.tile_pool(name="pcp2", bufs=2, space="PSUM") as pcp2:
            for t in range(NT):
                ps = pcp2.tile([P, 1], F32, tag="pc2")
                for k in range(KC):
                    nc.tensor.matmul(ps[:], qT32_t[:, k, t * P:(t + 1) * P],
                                     wc32_t[:, k, :], start=(k == 0), stop=(k == KC - 1))
                nc.scalar.activation(pc2[:, t:t + 1], ps[:], AF.Sigmoid,
                                     bias=bc_t[:])

        with tc.tile_pool(name="winp", bufs=2) as winp, \
             tc.tile_pool(name="qinps", bufs=1, space="PSUM") as qinps:
            qps = qinps.tile([P, KC, 512], F32)  # one bank per d-group
            for k in range(KC):
                wk = winp.tile([P, D], F32, tag="wk")
                nc.sync.dma_start(wk[:], w_in_r[:, k, :])
                for d in range(KC):
                    nc.tensor.matmul(qps[:, d, :TPC], wk[:, d * P:(d + 1) * P],
                                     qT32_t[:, k, :],
                                     start=(k == 0), stop=(k == KC - 1))
            nc.scalar.copy(qint[:], qps[:, :, :TPC])

        with tc.tile_pool(name="memp", bufs=3) as memp, \
             tc.tile_pool(name="aps", bufs=1, space="PSUM") as apsum, \
             tc.tile_pool(name="scb", bufs=1) as scb:
            sc_ps = [apsum.tile([P, S], F32, tag=f"sc{t}", name=f"sc_ps{t}")
                     for t in range(NT)]
            for k in range(KC):
                mk = memp.tile([P, S], F32, tag="mk")
                nc.sync.dma_start(mk[:], memT_r[:, k, :])
                for t in range(NT):
                    nc.tensor.matmul(sc_ps[t][:], qint[:, k, t * P:(t + 1) * P],
                                     mk[:], start=(k == 0), stop=(k == KC - 1))
            for t in range(NT):
                scores = scb.tile([P, S], F32, tag=f"scores{t}", name=f"scores{t}")
                negmax = small.tile([P, 1], F32, tag=f"negmax{t}", name=f"negmax{t}")
                nc.vector.tensor_add(scores[:], sc_ps[t][:], mask_t[:])
                nc.vector.tensor_reduce(negmax[:], scores[:], op=ALU.max,
                                        axis=mybir.AxisListType.X)
                nc.vector.tensor_scalar_mul(negmax[:], negmax[:], -1.0)
                nc.scalar.activation(attn16[:, t, :], scores[:], AF.Exp,
                                     bias=negmax[:],
                                     accum_out=asum[:, t:t + 1])
            for t in range(NT):
                for sc in range(S // P):
                    tp = apsum.tile([P, P], F16, tag="tp", name="tp")
                    nc.tensor.transpose(tp[:], attn16[:, t, sc * P:(sc + 1) * P],
                                        ident[:])
                    nc.scalar.copy(attnT[:, sc, t * P:(t + 1) * P], tp[:])
            for t in range(NT):
                cps = apsum.tile([P, E], F32, tag=f"cp{t}", name=f"cps{t}")
                for sc in range(S // P):
                    nc.tensor.matmul(cps[:], attnT[:, sc, t * P:(t + 1) * P],
                                     smap_t[:, sc, :],
                                     start=(sc == 0), stop=(sc == S // P - 1))
                nc.vector.reciprocal(c2[:, t:t + 1], asum[:, t:t + 1])
                nc.vector.tensor_mul(c2[:, t:t + 1], c2[:, t:t + 1], pc2[:, t:t + 1])
                cob = scb.tile([P, E], F32, tag=f"co{t}", name=f"cob{t}")
                nc.vector.tensor_scalar_mul(cob[:], cps[:], c2[:, t:t + 1])
                nc.sync.dma_start(copy_out.ap()[t * P:(t + 1) * P, :], cob[:])

    nc.compile()
    return nc


_CACHE = {}


def _get_nc(has_bgen: bool):
    if has_bgen not in _CACHE:
        _CACHE[has_bgen] = build(has_bgen)
    return _CACHE[has_bgen]


def _prep_in_maps(query, memory_bank, src_pad_mask, src_map, W_in, W_copy,
                  b_copy, W_gen, b_gen):
    w16 = np.ascontiguousarray(W_gen, dtype=np.float32).astype(np.float16)
    w16[:, 0] = 0.0  # PAD column: exp(0) == exp(-1e-20) == 1.0f
    has_bgen = bool(np.any(b_gen))
    w_in = np.ascontiguousarray(W_in, dtype=np.float32)
    w_copy = np.ascontiguousarray(W_copy, dtype=np.float32).reshape(D, 1)
    wc16_ = w_copy.astype(np.float16)
    b_copy = np.ascontiguousarray(b_copy, dtype=np.float32).reshape(1)
    bgen16 = np.asarray(b_gen).astype(np.float16)

    qf = np.ascontiguousarray(query, dtype=np.float32).reshape(NTOK, D)
    qT_all = np.ascontiguousarray(qf.T)            # [D, 2048] f32
    qT16a = qT_all.astype(np.float16)

    in_maps = []
    for c in range(8):
        b, h = c // 2, c % 2
        qT = np.ascontiguousarray(qf[c * TPC:(c + 1) * TPC].T)
        m = {
            "qT16a": qT16a,
            "qT32": qT,
            "w16s": np.ascontiguousarray(w16[:, c * VS:(c + 1) * VS]),
            "wc16": wc16_,
            "w_copy": w_copy,
            "b_copy": b_copy,
            "w_in": w_in,
            "memT": np.ascontiguousarray(memory_bank[b].T, dtype=np.float32),
            "smap": np.ascontiguousarray(src_map[b], dtype=np.float32).astype(np.float16),
            "maskadd": np.where(src_pad_mask[b], np.float32(NEG_INF),
                                np.float32(0.0)).astype(np.float32),
        }
        if has_bgen:
            m["bgen"] = np.ascontiguousarray(bgen16[c * VS:(c + 1) * VS])
        in_maps.append(m)
    return in_maps, has_bgen


def run(trace=False, trace_cores=None, **inputs):
    in_maps, has_bgen = _prep_in_maps(**{
        k: np.asarray(v) for k, v in inputs.items()})
    nc = _get_nc(has_bgen)
    kw = {}
    if trace:
        kw = dict(trace=True, trace_cores=trace_cores or [0])
    res = run_bass_kernel_spmd(nc, in_maps, core_ids=list(range(8)), **kw)
    gen = np.concatenate([res.results[c]["gen_out"] for c in range(8)], axis=1)
    cp = np.concatenate([res.results[c]["copy_out"] for c in range(8)], axis=0)
    out = np.concatenate([gen, cp], axis=1).reshape(B, T, V + E)
    return np.ascontiguousarray(out, dtype=np.float32), res


def kernel(**inputs):
    out, _ = run(**inputs)
    return out
